# revision 26
# baseline (speedup 1.0000x reference)
"""Depthwise 4x4 binomial blur on (16, 256, 128, 128) f32 across 8 TRN2 cores.

Filter: k = outer(g, g), g = [1,3,3,1]/8, pad (2,1) both spatial dims.

v4 design ("P2Q", fp16 I/O) — every engine under the DMA roofline:

  W-conv first, split as  8*Wconv(x) = p + 3q  with
      p_j = x_{j-2} + x_{j+1}      (plain adds, DVE tensor_tensor, 2x 16-bit)
      q_j = x_{j-1} + x_j
  computed in ONE DVE instruction per group: a 3-dim AP whose middle dim
  steps +1 cols on one operand and -1 on the other yields both p (s=0) and
  q (s=1) halves of a [128, 4096] pq tile.  [DVE ~74us]
  H-conv + scales on the PE as TWO accumulated matmuls per PSUM bank:
      out = (B/64) @ p + (3B/64) @ q,   B banded [1,3,3,1] (128x128)
  [PE ~55us busy]
  PSUM f32 -> fp16 evacuation on the Scalar (ACT) engine (activation Copy),
  which the HWDGE out-ring doesn't occupy (DMA triggers free the engine).
  [ACT ~63us]

  (A "Plan D" variant moving the q half onto the PE as strided in-tile
  passes measured 109.5us vs this design's ~98us: the extra PE passes
  stall on PSUM turnaround and contend with DMA writes into SBUF.)

  fp16 everywhere: halves DMA traffic vs f32 (the problem is memory-bound);
  weights {1,3,9}/64 are exact in fp16; measured rel err ~5e-4 (gate 2e-2).

  DMA layout: host prepacks quad-images so every DMA descriptor is a
  contiguous 16.8KB (in) / 16.4KB (out) per-partition row (measured rates:
  512B 13GB/s/engine, 4KB 23.5, 8KB 25.9, 16KB 26.7, 32KB 26.9).  Input planes sit at a 131-col stride with 3 zero cols between
  planes, so the shifted p/q reads see the conv zero-padding for free.
  (A dense 8192B-aligned input layout with gpsimd boundary-fix copies
  measured 121us: the tiny Q7 copies cost ~0.8us each and serialize the
  premix->matmul edge.  The 2.5% gap bytes are cheaper.)

  Per core: 17.2MB in + 16.8MB out = 34MB at ~400GB/s aggregate -> ~84.5us
  of saturated DMA + ~9us framework startup + ~2.5us drain.
  Engine busy: DVE ~74us, ACT ~63us, PE ~55us (all inside the DMA window).
  In/out transfers MUST ride separate HWDGE rings (sync=in, scalar=out):
  one queue runs a single transfer at a time, so a lone ring serializes
  in+out (measured 124us vs 96us).

Sharding: pure data-parallel, batch dim 16 -> 2 batches (512 planes) per core.
"""

import numpy as np

import concourse.bass as bass
import concourse.mybir as mybir
from concourse.tile import TileContext
from concourse.bass_utils import run_bass_kernel_spmd

B, C, H, W = 16, 256, 128, 128
N_CORES = 8
PLANES_PER_CORE = (B // N_CORES) * C  # 512
G = 16                 # planes per group (one PSUM double-buffer half)
N_GROUPS = PLANES_PER_CORE // G       # 32
N_QUADS = N_GROUPS // 4               # 8 (four groups per DMA)
STRIDE = W + 3         # 131: plane stride in in-tile cols, 3 zero cols between
LEAD = 3               # zero cols before plane 0 (shift -2 needs 2)
GRP_W = LEAD + STRIDE * G + 1         # 2100: one group's image width (cols)
QUAD_W = 4 * GRP_W                    # 8400 cols = 16.8 KB rows
OUT_QW = 4 * G * W                    # 8192 cols = 16.4 KB rows
NB_IN = 4              # in quad-tiles
NB_PQ = 6              # p premix tiles
NB_OUT = 3             # out quad-tiles


def _wpq_np():
    """lhsT weights [128, 256] fp16: cols 0:128 = (B/64).T, 128:256 = (3B/64).T
    with B[i, h] = b[h - i + 2], b = [1,3,3,1], truncated at edges."""
    b = np.array([1.0, 3.0, 3.0, 1.0], np.float64)
    Bm = np.zeros((H, H))
    for i in range(H):
        for d in range(4):
            h = i + d - 2
            if 0 <= h < H:
                Bm[i, h] = b[d]
    w = np.zeros((H, 2 * H), np.float16)
    w[:, 0:H] = (Bm / 64.0).T.astype(np.float16)
    w[:, H : 2 * H] = (3.0 * Bm / 64.0).T.astype(np.float16)
    return w


def _split_excess_waits(nc, max_waits=1):
    """TRN2 ISA instructions carry at most one sync-wait; hoist all-but-one
    wait onto fresh NOPs inserted immediately before the instruction on the
    same engine (program order preserved -> semantics unchanged)."""
    f = nc.m.functions[0]
    for blk in f.blocks:
        insts = blk.instructions  # live list; in-place edits persist
        i = 0
        while i < len(insts):
            inst = insts[i]
            si = getattr(inst, "sync_info", None)
            if si is not None and si.on_wait and len(si.on_wait) > max_waits:
                waits = list(si.on_wait)
                keep, extra = waits[-max_waits:], waits[:-max_waits]
                nops = []
                for k, wt in enumerate(extra):
                    n = mybir.InstNoOp(
                        name=f"{inst.name}-wsplit-{k}",
                        engine=inst.engine,
                        sync_info=mybir.SyncInfo(on_wait=[wt], on_update=[]),
                    )
                    nc.register_instruction(n)
                    nops.append(n)
                inst.sync_info = mybir.SyncInfo(
                    on_wait=keep, on_update=list(si.on_update)
                )
                insts[i:i] = nops
                i += len(nops)
            i += 1


def build_nc():
    nc = bass.Bass()
    dt = mybir.dt
    f16 = dt.float16

    x0_ext = nc.declare_dram_parameter(
        "x0", [N_QUADS, H, QUAD_W], f16, isOutput=False
    )  # prepadded quad-images: planes at 131-col stride, zero gaps between
    w_ext = nc.declare_dram_parameter("w", [H, 2 * H], f16, isOutput=False)
    out_ext = nc.declare_dram_parameter(
        "out", [N_QUADS, H, OUT_QW], f16, isOutput=True
    )

    with TileContext(nc) as tc:
        with (
            tc.tile_pool(name="wp", bufs=1) as wp,
            tc.tile_pool(name="io", bufs=1) as io,
            tc.tile_pool(name="ps", bufs=1, space="PSUM") as pp,
        ):
            w_sb = wp.tile([H, 2 * H], f16, tag="w", name="w_sb")
            # weights on the scalar ring: sync fires the pair-0 load first
            nc.scalar.dma_start(out=w_sb[:], in_=w_ext[:])

            in_tiles = [
                io.tile([H, QUAD_W], f16, tag=f"in{j}", name=f"in{j}")
                for j in range(NB_IN)
            ]
            pq_tiles = [
                io.tile([H, 2 * G * W], f16, tag=f"pq{j}", name=f"pq{j}")
                for j in range(NB_PQ)
            ]
            out_tiles = [
                io.tile([H, OUT_QW], f16, tag=f"out{j}", name=f"out{j}")
                for j in range(NB_OUT)
            ]
            ps_tiles = [
                pp.tile([H, G * W], dt.float32, tag=f"ps{j}", name=f"ps{j}")
                for j in range(2)
            ]

            for g in range(N_GROUPS):
                quad, qh = g // 4, g % 4
                it = in_tiles[quad % NB_IN]
                pqt = pq_tiles[g % NB_PQ]
                ot = out_tiles[quad % NB_OUT]
                ps = ps_tiles[g % 2]

                if qh == 0:
                    if quad == 0:
                        # split the first load (1 group + 3 groups) so group
                        # 0's premix starts a GRP-sized DMA into the fill
                        for a, b in ((0, 1), (1, 4)):
                            nc.sync.dma_start(
                                out=it[:, a * GRP_W : b * GRP_W],
                                in_=x0_ext[0, :, a * GRP_W : b * GRP_W],
                            )
                    else:
                        nc.sync.dma_start(out=it[:], in_=x0_ext[quad])

                # plane p data at col qh*GRP_W + LEAD + STRIDE*p; zero gaps
                # make the shifted reads see conv zero-padding.
                base = qh * GRP_W + LEAD

                # fused premix, ONE DVE instruction: out[h, s, pl, j] with
                # s=0 -> p_j = x_{j-2} + x_{j+1} (cols base-2 / base+1)
                # s=1 -> q_j = x_{j-1} + x_j    (cols base-1 / base+0)
                # via +1 / -1 strides on the s dim of the two operands.
                in0 = bass.AP(
                    it[:].tensor, base - 2, [[QUAD_W, H], [1, 2], [STRIDE, G], [1, W]]
                )
                in1 = bass.AP(
                    it[:].tensor, base + 1, [[QUAD_W, H], [-1, 2], [STRIDE, G], [1, W]]
                )
                pq_out = bass.AP(
                    pqt[:].tensor, 0, [[2 * G * W, H], [G * W, 2], [W, G], [1, W]]
                )
                nc.vector.tensor_add(out=pq_out, in0=in0, in1=in1)

                # H-conv: ps[:, bank] = (B/64)@p + (3B/64)@q, 4 banks of 512
                for b4 in range(4):
                    sl = slice(512 * b4, 512 * (b4 + 1))
                    nc.tensor.matmul(
                        out=ps[:, sl],
                        lhsT=w_sb[:, 0:H],
                        rhs=pqt[:, sl],
                        start=True,
                        stop=False,
                        skip_group_check=True,
                    )
                for b4 in range(4):
                    sl = slice(512 * b4, 512 * (b4 + 1))
                    nc.tensor.matmul(
                        out=ps[:, sl],
                        lhsT=w_sb[:, H : 2 * H],
                        rhs=pqt[:, G * W + 512 * b4 : G * W + 512 * (b4 + 1)],
                        start=False,
                        stop=True,
                        skip_group_check=True,
                    )

                # PSUM f32 -> fp16 evacuation on ACT
                obase = qh * G * W
                if g == N_GROUPS - 1:
                    # tail trim: evacuate + ship the last group in halves so
                    # the final store starts as early as possible
                    for hh in range(2):
                        hsl = slice(obase + 1024 * hh, obase + 1024 * (hh + 1))
                        nc.scalar.activation(
                            out=ot[:, hsl],
                            in_=ps[:, 1024 * hh : 1024 * (hh + 1)],
                            func=mybir.ActivationFunctionType.Copy,
                        )
                        nc.scalar.dma_start(out=out_ext[quad, :, hsl], in_=ot[:, hsl])
                else:
                    nc.scalar.activation(
                        out=ot[:, obase : obase + G * W],
                        in_=ps[:],
                        func=mybir.ActivationFunctionType.Copy,
                    )
                    if quad == N_QUADS - 1:
                        # last quad: ship each group as its evac lands
                        nc.scalar.dma_start(
                            out=out_ext[quad, :, obase : obase + G * W],
                            in_=ot[:, obase : obase + G * W],
                        )
                    elif qh == 3:
                        nc.scalar.dma_start(out=out_ext[quad], in_=ot[:])

    _split_excess_waits(nc)
    return nc


_cached_nc = None


def _get_nc():
    global _cached_nc
    if _cached_nc is None:
        _cached_nc = build_nc()
    return _cached_nc


def _pack_inputs(x):
    """x [16,256,128,128] f32 -> per-core prepadded fp16 quad-images."""
    x16 = np.ascontiguousarray(x, dtype=np.float32).astype(np.float16)
    # core k gets batches [2k, 2k+1]; planes grouped 16 at a time
    xg = x16.reshape(N_CORES, N_QUADS, 4, G, H, W)
    x0 = np.zeros((N_CORES, N_QUADS, H, QUAD_W), np.float16)
    for qh in range(4):
        for p in range(G):
            col = qh * GRP_W + LEAD + STRIDE * p
            x0[:, :, :, col : col + W] = xg[:, :, qh, p]
    return x0


def _unpack_output(res):
    """per-core [8,128,8192] fp16 -> [16,256,128,128] f32."""
    outs = np.stack([res.results[k]["out"] for k in range(N_CORES)])
    o = outs.reshape(N_CORES, N_QUADS, H, 4, G, W)
    o = o.transpose(0, 1, 3, 4, 2, 5)  # [cores, quads, 4, G, H, W]
    return o.reshape(B, C, H, W).astype(np.float32)


def _run(x, **spmd_kwargs):
    assert x.shape == (B, C, H, W), x.shape
    x0 = _pack_inputs(x)
    w = _wpq_np()
    in_maps = [{"x0": x0[k], "w": w} for k in range(N_CORES)]
    res = run_bass_kernel_spmd(_get_nc(), in_maps, list(range(N_CORES)), **spmd_kwargs)
    return _unpack_output(res), res


def kernel(x):
    out, _ = _run(np.asarray(x))
    return out


# revision 27
# speedup vs baseline: 1.1224x; 1.1224x over previous
"""Depthwise 4x4 binomial blur on (16, 256, 128, 128) f32 across 8 TRN2 cores.

Filter: k = outer(g, g), g = [1,3,3,1]/8, pad (2,1) both spatial dims.

v4 design ("P2Q", fp16 I/O) — every engine under the DMA roofline:

  W-conv first, split as  8*Wconv(x) = p + 3q  with
      p_j = x_{j-2} + x_{j+1}      (plain adds, DVE tensor_tensor, 2x 16-bit)
      q_j = x_{j-1} + x_j
  computed in ONE DVE instruction per group: a 3-dim AP whose middle dim
  steps +1 cols on one operand and -1 on the other yields both p (s=0) and
  q (s=1) halves of a [128, 4096] pq tile.  [DVE ~74us]
  H-conv + scales on the PE as TWO accumulated matmuls per PSUM bank:
      out = (B/64) @ p + (3B/64) @ q,   B banded [1,3,3,1] (128x128)
  [PE ~55us busy]
  PSUM f32 -> fp16 evacuation on the Scalar (ACT) engine (activation Copy),
  which the HWDGE out-ring doesn't occupy (DMA triggers free the engine).
  [ACT ~63us]

  (A "Plan D" variant moving the q half onto the PE as strided in-tile
  passes measured 109.5us vs this design's ~98us: the extra PE passes
  stall on PSUM turnaround and contend with DMA writes into SBUF.)

  fp16 everywhere: halves DMA traffic vs f32 (the problem is memory-bound);
  weights {1,3,9}/64 are exact in fp16; measured rel err ~5e-4 (gate 2e-2).

  DMA layout: host prepacks quad-images so every DMA descriptor is a
  contiguous 16.8KB (in) / 16.4KB (out) per-partition row (measured rates:
  512B 13GB/s/engine, 4KB 23.5, 8KB 25.9, 16KB 26.7, 32KB 26.9).  Input planes sit at a 131-col stride with 3 zero cols between
  planes, so the shifted p/q reads see the conv zero-padding for free.
  (A dense 8192B-aligned input layout with gpsimd boundary-fix copies
  measured 121us: the tiny Q7 copies cost ~0.8us each and serialize the
  premix->matmul edge.  The 2.5% gap bytes are cheaper.)

  Per core: 17.2MB in + 16.8MB out = 34MB at ~400GB/s aggregate -> ~84.5us
  of saturated DMA + ~9us framework startup + ~2.5us drain.
  Engine busy: DVE ~74us, ACT ~63us, PE ~55us (all inside the DMA window).
  In/out transfers MUST ride separate HWDGE rings (sync=in, scalar=out):
  one queue runs a single transfer at a time, so a lone ring serializes
  in+out (measured 124us vs 96us).

Sharding: pure data-parallel, batch dim 16 -> 2 batches (512 planes) per core.
"""

import numpy as np

import concourse.bass as bass
import concourse.mybir as mybir
from concourse.tile import TileContext
from concourse.bass_utils import run_bass_kernel_spmd

B, C, H, W = 16, 256, 128, 128
N_CORES = 8
PLANES_PER_CORE = (B // N_CORES) * C  # 512
G = 16                 # planes per group (one PSUM double-buffer half)
N_GROUPS = PLANES_PER_CORE // G       # 32
N_QUADS = N_GROUPS // 4               # 8 (four groups per DMA)
STRIDE = W + 3         # 131: plane stride in in-tile cols, 3 zero cols between
LEAD = 3               # zero cols before plane 0 (shift -2 needs 2)
GRP_W = LEAD + STRIDE * G + 1         # 2100: one group's image width (cols)
QUAD_W = 4 * GRP_W                    # 8400 cols = 16.8 KB rows
OUT_QW = 4 * G * W                    # 8192 cols = 16.4 KB rows
NB_IN = 3              # in quad-tiles
NB_PQ = 4              # p premix tiles
NB_OUT = 3             # out quad-tiles


def _wpq_np():
    """lhsT weights [128, 256] fp16: cols 0:128 = (B/64).T, 128:256 = (3B/64).T
    with B[i, h] = b[h - i + 2], b = [1,3,3,1], truncated at edges."""
    b = np.array([1.0, 3.0, 3.0, 1.0], np.float64)
    Bm = np.zeros((H, H))
    for i in range(H):
        for d in range(4):
            h = i + d - 2
            if 0 <= h < H:
                Bm[i, h] = b[d]
    w = np.zeros((H, 2 * H), np.float16)
    w[:, 0:H] = (Bm / 64.0).T.astype(np.float16)
    w[:, H : 2 * H] = (3.0 * Bm / 64.0).T.astype(np.float16)
    return w


def _split_excess_waits(nc, max_waits=1):
    """TRN2 ISA instructions carry at most one sync-wait; hoist all-but-one
    wait onto fresh NOPs inserted immediately before the instruction on the
    same engine (program order preserved -> semantics unchanged)."""
    f = nc.m.functions[0]
    for blk in f.blocks:
        insts = blk.instructions  # live list; in-place edits persist
        i = 0
        while i < len(insts):
            inst = insts[i]
            si = getattr(inst, "sync_info", None)
            if si is not None and si.on_wait and len(si.on_wait) > max_waits:
                waits = list(si.on_wait)
                keep, extra = waits[-max_waits:], waits[:-max_waits]
                nops = []
                for k, wt in enumerate(extra):
                    n = mybir.InstNoOp(
                        name=f"{inst.name}-wsplit-{k}",
                        engine=inst.engine,
                        sync_info=mybir.SyncInfo(on_wait=[wt], on_update=[]),
                    )
                    nc.register_instruction(n)
                    nops.append(n)
                inst.sync_info = mybir.SyncInfo(
                    on_wait=keep, on_update=list(si.on_update)
                )
                insts[i:i] = nops
                i += len(nops)
            i += 1


def build_nc():
    nc = bass.Bass()
    dt = mybir.dt
    f16 = dt.float16

    x0_ext = nc.declare_dram_parameter(
        "x0", [N_QUADS, H, QUAD_W], f16, isOutput=False
    )  # prepadded quad-images: planes at 131-col stride, zero gaps between
    w_ext = nc.declare_dram_parameter("w", [H, 2 * H], f16, isOutput=False)
    out_ext = nc.declare_dram_parameter(
        "out", [N_QUADS, H, OUT_QW], f16, isOutput=True
    )

    with TileContext(nc) as tc:
        with (
            tc.tile_pool(name="wp", bufs=1) as wp,
            tc.tile_pool(name="io", bufs=1) as io,
            tc.tile_pool(name="ps", bufs=1, space="PSUM") as pp,
        ):
            w_sb = wp.tile([H, 2 * H], f16, tag="w", name="w_sb")
            # weights on the scalar ring: sync fires the pair-0 load first
            nc.scalar.dma_start(out=w_sb[:], in_=w_ext[:])

            in_tiles = [
                io.tile([H, QUAD_W], f16, tag=f"in{j}", name=f"in{j}")
                for j in range(NB_IN)
            ]
            pq_tiles = [
                io.tile([H, 2 * G * W], f16, tag=f"pq{j}", name=f"pq{j}")
                for j in range(NB_PQ)
            ]
            out_tiles = [
                io.tile([H, OUT_QW], f16, tag=f"out{j}", name=f"out{j}")
                for j in range(NB_OUT)
            ]
            ps_tiles = [
                pp.tile([H, G * W], dt.float32, tag=f"ps{j}", name=f"ps{j}")
                for j in range(2)
            ]

            for g in range(N_GROUPS):
                quad, qh = g // 4, g % 4
                it = in_tiles[quad % NB_IN]
                pqt = pq_tiles[g % NB_PQ]
                ot = out_tiles[quad % NB_OUT]
                ps = ps_tiles[g % 2]

                if qh == 0:
                    if quad == 0:
                        # split the first load so group 0's premix starts
                        # three quarter-DMAs earlier (pipeline fill)
                        for qq in range(4):
                            nc.sync.dma_start(
                                out=it[:, qq * GRP_W : (qq + 1) * GRP_W],
                                in_=x0_ext[0, :, qq * GRP_W : (qq + 1) * GRP_W],
                            )
                    else:
                        nc.sync.dma_start(out=it[:], in_=x0_ext[quad])

                # plane p data at col qh*GRP_W + LEAD + STRIDE*p; zero gaps
                # make the shifted reads see conv zero-padding.
                base = qh * GRP_W + LEAD

                # fused premix, ONE DVE instruction: out[h, s, pl, j] with
                # s=0 -> p_j = x_{j-2} + x_{j+1} (cols base-2 / base+1)
                # s=1 -> q_j = x_{j-1} + x_j    (cols base-1 / base+0)
                # via +1 / -1 strides on the s dim of the two operands.
                in0 = bass.AP(
                    it[:].tensor, base - 2, [[QUAD_W, H], [1, 2], [STRIDE, G], [1, W]]
                )
                in1 = bass.AP(
                    it[:].tensor, base + 1, [[QUAD_W, H], [-1, 2], [STRIDE, G], [1, W]]
                )
                pq_out = bass.AP(
                    pqt[:].tensor, 0, [[2 * G * W, H], [G * W, 2], [W, G], [1, W]]
                )
                nc.vector.tensor_add(out=pq_out, in0=in0, in1=in1)

                # H-conv: ps[:, bank] = (B/64)@p + (3B/64)@q, 4 banks of 512
                for b4 in range(4):
                    sl = slice(512 * b4, 512 * (b4 + 1))
                    nc.tensor.matmul(
                        out=ps[:, sl],
                        lhsT=w_sb[:, 0:H],
                        rhs=pqt[:, sl],
                        start=True,
                        stop=False,
                        skip_group_check=True,
                    )
                for b4 in range(4):
                    sl = slice(512 * b4, 512 * (b4 + 1))
                    nc.tensor.matmul(
                        out=ps[:, sl],
                        lhsT=w_sb[:, H : 2 * H],
                        rhs=pqt[:, G * W + 512 * b4 : G * W + 512 * (b4 + 1)],
                        start=False,
                        stop=True,
                        skip_group_check=True,
                    )

                # PSUM f32 -> fp16 evacuation on ACT
                obase = qh * G * W
                if g == N_GROUPS - 1:
                    # tail trim: evacuate + ship the last group in halves so
                    # the final store starts as early as possible
                    for hh in range(2):
                        hsl = slice(obase + 1024 * hh, obase + 1024 * (hh + 1))
                        nc.scalar.activation(
                            out=ot[:, hsl],
                            in_=ps[:, 1024 * hh : 1024 * (hh + 1)],
                            func=mybir.ActivationFunctionType.Copy,
                        )
                        nc.scalar.dma_start(out=out_ext[quad, :, hsl], in_=ot[:, hsl])
                else:
                    nc.scalar.activation(
                        out=ot[:, obase : obase + G * W],
                        in_=ps[:],
                        func=mybir.ActivationFunctionType.Copy,
                    )
                    if quad == N_QUADS - 1:
                        # last quad: ship each group as its evac lands
                        nc.scalar.dma_start(
                            out=out_ext[quad, :, obase : obase + G * W],
                            in_=ot[:, obase : obase + G * W],
                        )
                    elif qh == 3:
                        nc.scalar.dma_start(out=out_ext[quad], in_=ot[:])

    _split_excess_waits(nc)
    return nc


_cached_nc = None


def _get_nc():
    global _cached_nc
    if _cached_nc is None:
        _cached_nc = build_nc()
    return _cached_nc


def _pack_inputs(x):
    """x [16,256,128,128] f32 -> per-core prepadded fp16 quad-images."""
    x16 = np.ascontiguousarray(x, dtype=np.float32).astype(np.float16)
    # core k gets batches [2k, 2k+1]; planes grouped 16 at a time
    xg = x16.reshape(N_CORES, N_QUADS, 4, G, H, W)
    x0 = np.zeros((N_CORES, N_QUADS, H, QUAD_W), np.float16)
    for qh in range(4):
        for p in range(G):
            col = qh * GRP_W + LEAD + STRIDE * p
            x0[:, :, :, col : col + W] = xg[:, :, qh, p]
    return x0


def _unpack_output(res):
    """per-core [8,128,8192] fp16 -> [16,256,128,128] f32."""
    outs = np.stack([res.results[k]["out"] for k in range(N_CORES)])
    o = outs.reshape(N_CORES, N_QUADS, H, 4, G, W)
    o = o.transpose(0, 1, 3, 4, 2, 5)  # [cores, quads, 4, G, H, W]
    return o.reshape(B, C, H, W).astype(np.float32)


def _run(x, **spmd_kwargs):
    assert x.shape == (B, C, H, W), x.shape
    x0 = _pack_inputs(x)
    w = _wpq_np()
    in_maps = [{"x0": x0[k], "w": w} for k in range(N_CORES)]
    res = run_bass_kernel_spmd(_get_nc(), in_maps, list(range(N_CORES)), **spmd_kwargs)
    return _unpack_output(res), res


def kernel(x):
    out, _ = _run(np.asarray(x))
    return out
